# revision 1
# baseline (speedup 1.0000x reference)
"""RIENet loss kernel (keypoint/KNN MSE + global-align Huber-min loss) on 8 trn2 cores.

Sharding: core ci -> (b = ci // 4, n-chunk j = ci % 4).  Each core holds the full
tgt[b] (M=8192 points) and a 2048-column chunk of src_transformed[b] (N axis).
  loss_1 (min over M per src point): complete locally per core.
  loss_2 (min over N per tgt point): per-core partial min over its chunk;
          host min-reduces the 4 chunks per batch element.

Device kernel per core (v2 — bf16-split matmul, PE off the critical path):
  Q[m, n] = -2 t_m . s_n + ||s_n||^2 computed by one K=21 bf16 matmul:
  t and s are split 3-way into bf16 (hi/mid/lo, ~27 mantissa bits total) and
  the 6 dominant cross products are taken (error ~1e-6 absolute); ||s||^2 is
  split 3-way against ones-rows.  ||t_m||^2 stays fp32 and is folded in
  per-partition by scalar_tensor_tensor during the column-min accumulation:
    acc = min(Q + nt[m], acc)           (min over m-tiles, DVE, one pass)
    rowbuf[:, mi] = reduce_min(Q)       (min over n-chunk, DVE, one pass)
  rowbuf gets nt added at the end; acc is partition-min-reduced via PE
  transposes.  Tiny keypoint/KNN MSE losses run on-device on every core.
"""

import os
import numpy as np


def _ensure_path():
    try:
        import concourse  # noqa: F401
    except ImportError:
        import sys
        for p in ("/opt/trn_rl_repo", "/root/.axon_site/_ro/trn_rl_repo"):
            if os.path.isdir(p) and p not in sys.path:
                sys.path.insert(0, p)


_ensure_path()

import concourse.bass as bass  # noqa: E402
import concourse.bacc as bacc  # noqa: E402
import concourse.tile as tile  # noqa: E402
import concourse.mybir as mybir  # noqa: E402
from concourse.bass_utils import run_bass_kernel_spmd  # noqa: E402

F32 = mybir.dt.float32
BF16 = mybir.dt.bfloat16
AL = mybir.AluOpType
AF = mybir.ActivationFunctionType

MARGIN = 0.1
B, KP, KNN, N, M = 2, 256, 32, 8192, 8192
NCORES = 8
NSHARDS = NCORES // B          # 4 n-chunks per batch element
CHUNK = N // NSHARDS           # 2048
NJ = CHUNK // 512              # 4 psum banks per m-tile
MI = M // 128                  # 64 m-tiles
GT = M // 128                  # 64 groups in the [p, d, g] tgt layout
GS = CHUNK // 128              # 16 groups in the [p, d, g] src layout
K21 = 21
BIG = 3.0e38

_CACHE = {}


def _build():
    nc = bacc.Bacc("TRN2", target_bir_lowering=False, debug=False,
                   num_devices=NCORES)

    src = nc.dram_tensor("src", [3, CHUNK], F32, kind="ExternalInput")
    tgt = nc.dram_tensor("tgt", [3, M], F32, kind="ExternalInput")
    ident = nc.dram_tensor("ident", [128, 128], F32, kind="ExternalInput")
    kp_lhsT = nc.dram_tensor("kp_lhsT", [4, 2 * 3], F32, kind="ExternalInput")
    kp_rhs = nc.dram_tensor("kp_rhs", [4, 2 * KP], F32, kind="ExternalInput")
    tgt_kp = nc.dram_tensor("tgt_kp", [3, 2 * KP], F32, kind="ExternalInput")
    knn_src = nc.dram_tensor("knn_src", [128, 2 * 192], F32, kind="ExternalInput")
    knn_tgt = nc.dram_tensor("knn_tgt", [128, 2 * 192], F32, kind="ExternalInput")

    colmin_o = nc.dram_tensor("colmin", [128, CHUNK // 128], F32, kind="ExternalOutput")
    rowmin_o = nc.dram_tensor("rowmin", [128, MI], F32, kind="ExternalOutput")
    misc_o = nc.dram_tensor("misc", [128, 4], F32, kind="ExternalOutput")

    with tile.TileContext(nc) as tc:
        with (
            tc.tile_pool(name="const", bufs=1) as const,
            tc.tile_pool(name="sc", bufs=3) as sc,
        ):
            tA = const.tile([K21, M], BF16)       # lhsT rows
            sA = const.tile([K21, CHUNK], BF16)   # rhs rows
            acc = const.tile([128, CHUNK], F32)
            rowbuf = const.tile([128, MI], F32)
            nt_all = const.tile([128, GT], F32)   # ||t||^2, [p, mi]
            id_sb = const.tile([128, 128], F32)
            colmin_sb = const.tile([128, CHUNK // 128], F32)
            misc_sb = const.tile([128, 4], F32)

            nc.sync.dma_start(out=id_sb[:], in_=ident[:])
            nc.gpsimd.memset(acc[:], BIG)
            nc.gpsimd.memset(misc_sb[:], 0.0)

            # ---- load t, s in [p, d, g] layouts (partition-minor DMA) ----
            tw = const.tile([128, 3, GT], F32)
            sw = const.tile([128, 3, GS], F32)
            nc.sync.dma_start(out=tw[:], in_=tgt.rearrange("d (g p) -> p d g", p=128))
            nc.sync.dma_start(out=sw[:], in_=src.rearrange("d (g p) -> p d g", p=128))

            # ---- norms (fp32) ----
            tsq = const.tile([128, 3, GT], F32)
            nc.vector.tensor_mul(tsq[:], tw[:], tw[:])
            nc.vector.tensor_add(nt_all[:], tsq[:, 0, :], tsq[:, 1, :])
            nc.vector.tensor_add(nt_all[:], nt_all[:], tsq[:, 2, :])
            ssq = const.tile([128, 3, GS], F32)
            ns_w = const.tile([128, GS], F32)
            nc.vector.tensor_mul(ssq[:], sw[:], sw[:])
            nc.vector.tensor_add(ns_w[:], ssq[:, 0, :], ssq[:, 1, :])
            nc.vector.tensor_add(ns_w[:], ns_w[:], ssq[:, 2, :])

            # ---- 3-way bf16 splits (kept as exactly-rounded fp32 tiles) ----
            nc.scalar.mul(out=tw[:], in_=tw[:], mul=-2.0)  # fold -2 into t side

            def split3(name, w, shape):
                outs = []
                cur = w
                for lvl in range(3):
                    b16 = sc.tile(shape, BF16, tag=f"{name}_b{lvl}")
                    nc.scalar.copy(out=b16[:], in_=cur[:])
                    f32t = const.tile(shape, F32, tag=f"{name}_f{lvl}")
                    nc.vector.tensor_copy(out=f32t[:], in_=b16[:])
                    outs.append(f32t)
                    if lvl < 2:
                        nxt = const.tile(shape, F32, tag=f"{name}_r{lvl}")
                        nc.vector.tensor_sub(nxt[:], cur[:], f32t[:])
                        cur = nxt
                return outs

            th, tm, tl = split3("t", tw, [128, 3, GT])
            sh, sm, sl = split3("s", sw, [128, 3, GS])
            nsp = split3("n", ns_w.rearrange("p (o g) -> p o g", o=1),
                         [128, 1, GS])

            # pairing layout: lhsT rows [th,th,tm,tm,th,tl]*3d + ones*3
            #                 rhs  rows [sh,sm,sh,sm,sl,sh]*3d + ns_splits
            t_dest = {0: [0, 3, 12], 1: [6, 9], 2: [15]}    # th, tm, tl
            s_dest = {0: [0, 6, 15], 1: [3, 9], 2: [12]}    # sh, sm, sl

            with tc.tile_pool(name="psum_pre", bufs=4, space="PSUM") as pp:
                def place(w_f32, groups, dst_tile, rows, width):
                    # transpose [128, g] -> [g, 128] via PE, cast to bf16,
                    # then DMA into row(s) of the operand tile
                    pt = pp.tile([groups, 128], F32, tag=f"tp{groups}")
                    nc.tensor.transpose(pt[:], w_f32, id_sb[:])
                    tr = sc.tile([groups, 128], BF16, tag=f"tr{groups}")
                    nc.scalar.copy(out=tr[:], in_=pt[:])
                    for r in rows:
                        nc.sync.dma_start(
                            out=dst_tile[r:r + 1, :].rearrange(
                                "o (g p) -> o g p", p=128),
                            in_=tr[:])

                for lvl, w in enumerate([th, tm, tl]):
                    for d in range(3):
                        place(w[:, d, :], GT, tA,
                              [base + d for base in t_dest[lvl]], M)
                for lvl, w in enumerate([sh, sm, sl]):
                    for d in range(3):
                        place(w[:, d, :], GS, sA,
                              [base + d for base in s_dest[lvl]], CHUNK)
                for lvl in range(3):
                    place(nsp[lvl][:, 0, :], GS, sA, [18 + lvl], CHUNK)

                # ones rows 18-20 of lhsT (staged at partition 0, DMA'd up)
                ones3 = const.tile([3, M], BF16)
                nc.vector.memset(ones3[:], 1.0)
                nc.sync.dma_start(out=tA[18:21, :], in_=ones3[:])

            # ---- main loop: Q = -2 t.s + ||s||^2 per 128-row m-tile ----
            with tc.tile_pool(name="psum_main", bufs=2, space="PSUM") as pm:
                for mi in range(MI):
                    pt = pm.tile([128, CHUNK], F32, tag="pt")
                    for nj in range(NJ):
                        nc.tensor.matmul(
                            pt[:, nj * 512:(nj + 1) * 512],
                            lhsT=tA[:, mi * 128:(mi + 1) * 128],
                            rhs=sA[:, nj * 512:(nj + 1) * 512],
                            start=True, stop=True,
                        )
                    # colmin: acc = min(Q + nt[m], acc)
                    nc.vector.scalar_tensor_tensor(
                        out=acc[:], in0=pt[:], scalar=nt_all[:, mi:mi + 1],
                        in1=acc[:], op0=AL.add, op1=AL.min)
                    # rowmin over the n-chunk (nt added after the loop)
                    nc.vector.tensor_reduce(
                        out=rowbuf[:, mi:mi + 1], in_=pt[:],
                        axis=mybir.AxisListType.X, op=AL.min)

            nc.vector.tensor_add(rowbuf[:], rowbuf[:], nt_all[:])

            with tc.tile_pool(name="psum_fin", bufs=2, space="PSUM") as pf:
                # partition-axis min of acc via PE transposes
                for blk in range(CHUNK // 128):
                    tp = pf.tile([128, 128], F32, tag="tp")
                    nc.tensor.transpose(tp[:], acc[:, blk * 128:(blk + 1) * 128],
                                        id_sb[:])
                    nc.vector.tensor_reduce(
                        out=colmin_sb[:, blk:blk + 1], in_=tp[:],
                        axis=mybir.AxisListType.X, op=AL.min)

                # tiny keypoint / knn losses (both batch elements)
                kp_l = const.tile([4, 2 * 3], F32)
                kp_r = const.tile([4, 2 * KP], F32)
                kp_t = const.tile([3, 2 * KP], F32)
                ks = const.tile([128, 2 * 192], F32)
                kt = const.tile([128, 2 * 192], F32)
                nc.sync.dma_start(out=kp_l[:], in_=kp_lhsT[:])
                nc.sync.dma_start(out=kp_r[:], in_=kp_rhs[:])
                nc.sync.dma_start(out=kp_t[:], in_=tgt_kp[:])
                nc.sync.dma_start(out=ks[:], in_=knn_src[:])
                nc.sync.dma_start(out=kt[:], in_=knn_tgt[:])
                for b in range(B):
                    pt2 = pf.tile([3, KP], F32, tag="kp")
                    nc.tensor.matmul(
                        pt2[:], lhsT=kp_l[:, b * 3:(b + 1) * 3],
                        rhs=kp_r[:, b * KP:(b + 1) * KP],
                        start=True, stop=True)
                    diff = sc.tile([3, KP], F32, tag="kdiff")
                    nc.vector.tensor_sub(diff[:], pt2[:],
                                         kp_t[:, b * KP:(b + 1) * KP])
                    nc.vector.tensor_mul(diff[:], diff[:], diff[:])
                    nc.vector.tensor_reduce(
                        out=misc_sb[0:3, b:b + 1], in_=diff[:],
                        axis=mybir.AxisListType.X, op=AL.add)
                    diff2 = sc.tile([128, 192], F32, tag="ndiff")
                    nc.vector.tensor_sub(diff2[:], ks[:, b * 192:(b + 1) * 192],
                                         kt[:, b * 192:(b + 1) * 192])
                    nc.vector.tensor_mul(diff2[:], diff2[:], diff2[:])
                    nc.vector.tensor_reduce(
                        out=misc_sb[:, 2 + b:3 + b], in_=diff2[:],
                        axis=mybir.AxisListType.X, op=AL.add)

            nc.sync.dma_start(out=colmin_o[:], in_=colmin_sb[:])
            nc.sync.dma_start(out=rowmin_o[:], in_=rowbuf[:])
            nc.sync.dma_start(out=misc_o[:], in_=misc_sb[:])

    nc.compile()
    return nc


def _get_nc():
    if "nc" not in _CACHE:
        _CACHE["nc"] = _build()
    return _CACHE["nc"]


def _prepare_in_maps(src_keypoints, tgt_keypoints, rotation_ab, translation_ab,
                     src_keypoints_knn, tgt_keypoints_knn, src_transformed, tgt):
    f = np.float32
    st = np.ascontiguousarray(np.asarray(src_transformed, dtype=f))
    tg = np.ascontiguousarray(np.asarray(tgt, dtype=f))
    skp = np.asarray(src_keypoints, dtype=f)
    tkp = np.asarray(tgt_keypoints, dtype=f)
    rot = np.asarray(rotation_ab, dtype=f)
    tra = np.asarray(translation_ab, dtype=f)
    sknn = np.asarray(src_keypoints_knn, dtype=f)
    tknn = np.asarray(tgt_keypoints_knn, dtype=f)

    ident = np.eye(128, dtype=f)
    kp_lhsT = np.zeros((4, 2 * 3), dtype=f)
    kp_rhs = np.zeros((4, 2 * KP), dtype=f)
    tgt_kp = np.zeros((3, 2 * KP), dtype=f)
    knn_src = np.zeros((128, 2 * 192), dtype=f)
    knn_tgt = np.zeros((128, 2 * 192), dtype=f)
    for b in range(B):
        kp_lhsT[0:3, b * 3:(b + 1) * 3] = rot[b].T
        kp_lhsT[3, b * 3:(b + 1) * 3] = tra[b]
        kp_rhs[0:3, b * KP:(b + 1) * KP] = skp[b]
        kp_rhs[3, b * KP:(b + 1) * KP] = 1.0
        tgt_kp[:, b * KP:(b + 1) * KP] = tkp[b]
        knn_src[:, b * 192:(b + 1) * 192] = sknn[b].reshape(128, 192)
        knn_tgt[:, b * 192:(b + 1) * 192] = tknn[b].reshape(128, 192)

    shared = {
        "ident": ident, "kp_lhsT": kp_lhsT, "kp_rhs": kp_rhs,
        "tgt_kp": tgt_kp, "knn_src": knn_src, "knn_tgt": knn_tgt,
    }
    in_maps = []
    for ci in range(NCORES):
        b, j = divmod(ci, NSHARDS)
        m = dict(shared)
        m["src"] = np.ascontiguousarray(st[b, :, j * CHUNK:(j + 1) * CHUNK])
        m["tgt"] = tg[b]
        in_maps.append(m)
    return in_maps


def _huber(x, c):
    return np.where(x < c, 0.5 * x * x, c * x - 0.5 * c * c)


def _postprocess(results):
    c = np.float64(MARGIN)
    loss1 = np.float64(0.0)
    loss2 = np.float64(0.0)
    for b in range(B):
        rowmins = []
        for j in range(NSHARDS):
            r = results[b * NSHARDS + j]
            colmin = np.asarray(r["colmin"], dtype=np.float64).T.ravel()
            loss1 += _huber(colmin, c).sum()
            rowmins.append(np.asarray(r["rowmin"], dtype=np.float64).T.ravel())
        rm = np.minimum.reduce(rowmins)
        loss2 += _huber(rm, c).sum()
    gal = loss1 + loss2

    misc = np.asarray(results[0]["misc"], dtype=np.float64)
    kp_loss = (misc[0:3, 0].sum() + misc[0:3, 1].sum()) / B
    knn_loss = (misc[:, 2].sum() + misc[:, 3].sum()) / (B * KNN)
    ncl = knn_loss + kp_loss
    return np.float32(ncl), np.float32(gal)


def run_device(in_maps, **kw):
    nc = _get_nc()
    return run_bass_kernel_spmd(nc, in_maps, list(range(NCORES)), **kw)


def kernel(src_keypoints, tgt_keypoints, rotation_ab, translation_ab,
           src_keypoints_knn, tgt_keypoints_knn, k, src_transformed, tgt,
           **_unused):
    in_maps = _prepare_in_maps(src_keypoints, tgt_keypoints, rotation_ab,
                               translation_ab, src_keypoints_knn,
                               tgt_keypoints_knn, src_transformed, tgt)
    res = run_device(in_maps)
    return _postprocess(res.results)



# revision 3
# speedup vs baseline: 1.5329x; 1.5329x over previous
"""RIENet loss kernel (keypoint/KNN MSE + global-align Huber-min loss) on 8 trn2 cores.

Sharding: core ci -> (b = ci // 4, n-chunk j = ci % 4).  Each core holds the full
tgt[b] (M=8192 points) and a 2048-column chunk of src_transformed[b] (N axis).
  loss_1 (min over M per src point): complete locally per core.
  loss_2 (min over N per tgt point): per-core partial min over its chunk;
          host min-reduces the 4 chunks per batch element.

v3 — three-engine pipeline, all operand prep on host:
  Host builds lhsT [24, M] / rhs [24, CHUNK] bf16 factor matrices so one
  K=24 matmul yields Q[m, n] = ||t_m - s_n||^2 directly in PSUM f32
  (3-way bf16 splits for -2 t.s, plus split ||s||^2 and ||t||^2 rows
  against ones).  Per 128-row m-tile:
    PE:  4 matmuls of 512 cols -> PSUM           (~0.9 us)
    ACT: copy PSUM f32 -> SBUF fp16 (q16)         (~1.8 us)  <- bottleneck
    DVE: acc  = min(acc, q16)     2x-mode TT      (~1.1 us)
         rowbuf[:, mi] = reduce_min(q16)  4x-mode (~0.6 us)
  acc is partition-min-reduced via PE transposes at the end.  Tiny
  keypoint/KNN MSE losses run on-device on every core.
"""

import os
import numpy as np
import ml_dtypes


def _ensure_path():
    try:
        import concourse  # noqa: F401
    except ImportError:
        import sys
        for p in ("/opt/trn_rl_repo", "/root/.axon_site/_ro/trn_rl_repo"):
            if os.path.isdir(p) and p not in sys.path:
                sys.path.insert(0, p)


_ensure_path()

import concourse.bass as bass  # noqa: E402
import concourse.bacc as bacc  # noqa: E402
import concourse.tile as tile  # noqa: E402
import concourse.mybir as mybir  # noqa: E402
from concourse.bass_utils import run_bass_kernel_spmd  # noqa: E402

F32 = mybir.dt.float32
F16 = mybir.dt.float16
BF16 = mybir.dt.bfloat16
AL = mybir.AluOpType
BF = ml_dtypes.bfloat16

MARGIN = 0.1
B, KP, KNN, N, M = 2, 256, 32, 8192, 8192
NCORES = 8
NSHARDS = NCORES // B          # 4 n-chunks per batch element
CHUNK = N // NSHARDS           # 2048
NJ = CHUNK // 512              # 4 psum banks per m-tile
MI = M // 128                  # 64 m-tiles
K24 = 24
ACC_INIT = 60000.0             # > max possible distance^2, fp16-representable

_CACHE = {}


def _build():
    nc = bacc.Bacc("TRN2", target_bir_lowering=False, debug=False,
                   num_devices=NCORES)

    tA_d = nc.dram_tensor("tA", [K24, M], BF16, kind="ExternalInput")
    sA_d = nc.dram_tensor("sA", [K24, CHUNK], BF16, kind="ExternalInput")
    ident = nc.dram_tensor("ident", [128, 128], F16, kind="ExternalInput")
    kp_lhsT = nc.dram_tensor("kp_lhsT", [4, 2 * 3], F32, kind="ExternalInput")
    kp_rhs = nc.dram_tensor("kp_rhs", [4, 2 * KP], F32, kind="ExternalInput")
    tgt_kp = nc.dram_tensor("tgt_kp", [3, 2 * KP], F32, kind="ExternalInput")
    knn_src = nc.dram_tensor("knn_src", [128, 2 * 192], F32, kind="ExternalInput")
    knn_tgt = nc.dram_tensor("knn_tgt", [128, 2 * 192], F32, kind="ExternalInput")

    colmin_o = nc.dram_tensor("colmin", [128, CHUNK // 128], F32, kind="ExternalOutput")
    rowmin_o = nc.dram_tensor("rowmin", [128, MI], F16, kind="ExternalOutput")
    misc_o = nc.dram_tensor("misc", [128, 4], F32, kind="ExternalOutput")

    with tile.TileContext(nc) as tc:
        with (
            tc.tile_pool(name="const", bufs=1) as const,
            tc.tile_pool(name="sc", bufs=3) as sc,
        ):
            tA = const.tile([K24, M], BF16)
            sA = const.tile([K24, CHUNK], BF16)
            acc = const.tile([128, CHUNK], F16)
            rowbuf = const.tile([128, MI], F16)
            id_sb = const.tile([128, 128], F16)
            colmin_sb = const.tile([128, CHUNK // 128], F32)
            misc_sb = const.tile([128, 4], F32)

            nc.sync.dma_start(out=tA[:], in_=tA_d[:])
            nc.sync.dma_start(out=sA[:], in_=sA_d[:])
            nc.sync.dma_start(out=id_sb[:], in_=ident[:])
            nc.gpsimd.memset(acc[:], ACC_INIT)
            nc.gpsimd.memset(misc_sb[:], 0.0)

            # ---- main loop: Q = dist^2 per 128-row m-tile ----
            with tc.tile_pool(name="psum_main", bufs=2, space="PSUM") as pm:
                for mi in range(MI):
                    pt = pm.tile([128, CHUNK], F32, tag="pt")
                    for nj in range(NJ):
                        nc.tensor.matmul(
                            pt[:, nj * 512:(nj + 1) * 512],
                            lhsT=tA[:, mi * 128:(mi + 1) * 128],
                            rhs=sA[:, nj * 512:(nj + 1) * 512],
                            start=True, stop=True,
                        )
                    q16 = sc.tile([128, CHUNK], F16, tag="q16")
                    nc.scalar.copy(out=q16[:], in_=pt[:])
                    nc.vector.tensor_tensor(acc[:], q16[:], acc[:], AL.min)
                    nc.vector.tensor_reduce(
                        out=rowbuf[:, mi:mi + 1], in_=q16[:],
                        axis=mybir.AxisListType.X, op=AL.min)

            with tc.tile_pool(name="psum_fin", bufs=2, space="PSUM") as pf:
                # partition-axis min of acc via PE transposes
                for blk in range(CHUNK // 128):
                    tp = pf.tile([128, 128], F16, tag="tp")
                    nc.tensor.transpose(tp[:], acc[:, blk * 128:(blk + 1) * 128],
                                        id_sb[:])
                    nc.vector.tensor_reduce(
                        out=colmin_sb[:, blk:blk + 1], in_=tp[:],
                        axis=mybir.AxisListType.X, op=AL.min)

                # tiny keypoint / knn losses (both batch elements)
                kp_l = const.tile([4, 2 * 3], F32)
                kp_r = const.tile([4, 2 * KP], F32)
                kp_t = const.tile([3, 2 * KP], F32)
                ks = const.tile([128, 2 * 192], F32)
                kt = const.tile([128, 2 * 192], F32)
                nc.sync.dma_start(out=kp_l[:], in_=kp_lhsT[:])
                nc.sync.dma_start(out=kp_r[:], in_=kp_rhs[:])
                nc.sync.dma_start(out=kp_t[:], in_=tgt_kp[:])
                nc.sync.dma_start(out=ks[:], in_=knn_src[:])
                nc.sync.dma_start(out=kt[:], in_=knn_tgt[:])
                for b in range(B):
                    pt2 = pf.tile([3, KP], F32, tag="kp")
                    nc.tensor.matmul(
                        pt2[:], lhsT=kp_l[:, b * 3:(b + 1) * 3],
                        rhs=kp_r[:, b * KP:(b + 1) * KP],
                        start=True, stop=True)
                    diff = sc.tile([3, KP], F32, tag="kdiff")
                    nc.vector.tensor_sub(diff[:], pt2[:],
                                         kp_t[:, b * KP:(b + 1) * KP])
                    nc.vector.tensor_mul(diff[:], diff[:], diff[:])
                    nc.vector.tensor_reduce(
                        out=misc_sb[0:3, b:b + 1], in_=diff[:],
                        axis=mybir.AxisListType.X, op=AL.add)
                    diff2 = sc.tile([128, 192], F32, tag="ndiff")
                    nc.vector.tensor_sub(diff2[:], ks[:, b * 192:(b + 1) * 192],
                                         kt[:, b * 192:(b + 1) * 192])
                    nc.vector.tensor_mul(diff2[:], diff2[:], diff2[:])
                    nc.vector.tensor_reduce(
                        out=misc_sb[:, 2 + b:3 + b], in_=diff2[:],
                        axis=mybir.AxisListType.X, op=AL.add)

            nc.sync.dma_start(out=colmin_o[:], in_=colmin_sb[:])
            nc.sync.dma_start(out=rowmin_o[:], in_=rowbuf[:])
            nc.sync.dma_start(out=misc_o[:], in_=misc_sb[:])

    nc.compile()
    return nc


def _get_nc():
    if "nc" not in _CACHE:
        _CACHE["nc"] = _build()
    return _CACHE["nc"]


def _split3(x):
    h = x.astype(BF).astype(np.float32)
    r = x - h
    m = r.astype(BF).astype(np.float32)
    l = (r - m).astype(BF).astype(np.float32)
    return h, m, l


def _build_ops(t, s):
    # t (3, M), s (3, CHUNK) f32 -> tA [24, M], sA [24, CHUNK] bf16 with
    # sum_k tA[k, m] * sA[k, n] ~= ||t_m - s_n||^2
    tm2 = -2.0 * t
    nt = (t * t).sum(0)
    ns = (s * s).sum(0)
    th, tm, tl = _split3(tm2)
    sh, sm, sl = _split3(s)
    nth, ntm, ntl = _split3(nt)
    nsh, nsm, nsl = _split3(ns)
    tA = np.zeros((K24, t.shape[1]), np.float32)
    sA = np.zeros((K24, s.shape[1]), np.float32)
    pairs = [(th, sh), (th, sm), (tm, sh), (tm, sm), (th, sl), (tl, sh)]
    for pi, (ta, sa) in enumerate(pairs):
        for d in range(3):
            tA[pi * 3 + d] = ta[d]
            sA[pi * 3 + d] = sa[d]
    tA[18:21] = 1.0
    sA[18], sA[19], sA[20] = nsh, nsm, nsl
    tA[21], tA[22], tA[23] = nth, ntm, ntl
    sA[21:24] = 1.0
    return (np.ascontiguousarray(tA.astype(BF)),
            np.ascontiguousarray(sA.astype(BF)))


def _prepare_in_maps(src_keypoints, tgt_keypoints, rotation_ab, translation_ab,
                     src_keypoints_knn, tgt_keypoints_knn, src_transformed, tgt):
    f = np.float32
    st = np.ascontiguousarray(np.asarray(src_transformed, dtype=f))
    tg = np.ascontiguousarray(np.asarray(tgt, dtype=f))
    skp = np.asarray(src_keypoints, dtype=f)
    tkp = np.asarray(tgt_keypoints, dtype=f)
    rot = np.asarray(rotation_ab, dtype=f)
    tra = np.asarray(translation_ab, dtype=f)
    sknn = np.asarray(src_keypoints_knn, dtype=f)
    tknn = np.asarray(tgt_keypoints_knn, dtype=f)

    ident = np.eye(128, dtype=np.float16)
    kp_lhsT = np.zeros((4, 2 * 3), dtype=f)
    kp_rhs = np.zeros((4, 2 * KP), dtype=f)
    tgt_kp = np.zeros((3, 2 * KP), dtype=f)
    knn_src = np.zeros((128, 2 * 192), dtype=f)
    knn_tgt = np.zeros((128, 2 * 192), dtype=f)
    for b in range(B):
        kp_lhsT[0:3, b * 3:(b + 1) * 3] = rot[b].T
        kp_lhsT[3, b * 3:(b + 1) * 3] = tra[b]
        kp_rhs[0:3, b * KP:(b + 1) * KP] = skp[b]
        kp_rhs[3, b * KP:(b + 1) * KP] = 1.0
        tgt_kp[:, b * KP:(b + 1) * KP] = tkp[b]
        knn_src[:, b * 192:(b + 1) * 192] = sknn[b].reshape(128, 192)
        knn_tgt[:, b * 192:(b + 1) * 192] = tknn[b].reshape(128, 192)

    shared = {
        "ident": ident, "kp_lhsT": kp_lhsT, "kp_rhs": kp_rhs,
        "tgt_kp": tgt_kp, "knn_src": knn_src, "knn_tgt": knn_tgt,
    }
    in_maps = []
    for ci in range(NCORES):
        b, j = divmod(ci, NSHARDS)
        m = dict(shared)
        tA, sA = _build_ops(tg[b], st[b][:, j * CHUNK:(j + 1) * CHUNK])
        m["tA"] = tA
        m["sA"] = sA
        in_maps.append(m)
    return in_maps


def _huber(x, c):
    return np.where(x < c, 0.5 * x * x, c * x - 0.5 * c * c)


def _postprocess(results):
    c = np.float64(MARGIN)
    loss1 = np.float64(0.0)
    loss2 = np.float64(0.0)
    for b in range(B):
        rowmins = []
        for j in range(NSHARDS):
            r = results[b * NSHARDS + j]
            colmin = np.asarray(r["colmin"], dtype=np.float64).T.ravel()
            loss1 += _huber(colmin, c).sum()
            rowmins.append(np.asarray(r["rowmin"], dtype=np.float64).T.ravel())
        rm = np.minimum.reduce(rowmins)
        loss2 += _huber(rm, c).sum()
    gal = loss1 + loss2

    misc = np.asarray(results[0]["misc"], dtype=np.float64)
    kp_loss = (misc[0:3, 0].sum() + misc[0:3, 1].sum()) / B
    knn_loss = (misc[:, 2].sum() + misc[:, 3].sum()) / (B * KNN)
    ncl = knn_loss + kp_loss
    return np.float32(ncl), np.float32(gal)


def run_device(in_maps, **kw):
    nc = _get_nc()
    return run_bass_kernel_spmd(nc, in_maps, list(range(NCORES)), **kw)


def kernel(src_keypoints, tgt_keypoints, rotation_ab, translation_ab,
           src_keypoints_knn, tgt_keypoints_knn, k, src_transformed, tgt,
           **_unused):
    in_maps = _prepare_in_maps(src_keypoints, tgt_keypoints, rotation_ab,
                               translation_ab, src_keypoints_knn,
                               tgt_keypoints_knn, src_transformed, tgt)
    res = run_device(in_maps)
    return _postprocess(res.results)
